# revision 2
# baseline (speedup 1.0000x reference)
"""Trainium2 Bass kernel for FastHoloLinear.

    resonance = x @ basis.T          # [B, H]
    out       = resonance @ (amp * cos(phase)).T   # [B, O]

Sharding: data-parallel over the batch dim across 8 NeuronCores; the small
basis/phase/amp parameters are replicated.

Per-core device program (B = 1024 rows/core):
  - GEMM1 in float32r (TF32-like, full PE rate; inputs are DMA'd directly
    as f32r so there is no extra rounding pass), contraction over IN_F
    accumulated in PSUM across 32 k-tiles.
  - w = amp * cos(phase) computed on-chip: ScalarE Sin LUT (bias=pi/2) +
    VectorE multiply, overlapped with GEMM1's x streaming.
  - GEMM2 in full fp32 (contraction dim is a single 128 tile, PE has slack).

Host side only reshapes/transposes for layout and gathers the shards.
"""

import math
from contextlib import ExitStack

import numpy as np

import concourse.bass as bass
import concourse.tile as tile
from concourse import bacc, mybir
from concourse.bass_utils import run_bass_kernel_spmd

F32 = mybir.dt.float32
F32R = mybir.dt.float32r

N_CORES = 8
B_FULL, IN_F, OUT_F, HARM = 8192, 4096, 4096, 128
B = B_FULL // N_CORES          # 1024 rows per core
P = 128                        # partition dim
KT = IN_F // P                 # 32 contraction tiles
KG = 4                         # k-tiles per x DMA (2 MiB transfers)
NG = KT // KG                  # 8 x-load groups
NCHUNK = 512                   # matmul moving free dim (one PSUM bank fp32)
BC = B // NCHUNK               # 2 batch chunks in GEMM1
BT = B // P                    # 8 batch tiles in GEMM2
OC = OUT_F // NCHUNK           # 8 output-column chunks in GEMM2


def _build():
    nc = bacc.Bacc("TRN2", target_bir_lowering=False, debug=False)

    xt_d = nc.dram_tensor("xt", [IN_F, B], F32R, kind="ExternalInput").ap()
    basist_d = nc.dram_tensor("basist", [IN_F, HARM], F32R, kind="ExternalInput").ap()
    phaset_d = nc.dram_tensor("phaset", [HARM, OUT_F], F32, kind="ExternalInput").ap()
    ampt_d = nc.dram_tensor("ampt", [HARM, OUT_F], F32, kind="ExternalInput").ap()
    out_d = nc.dram_tensor("out", [B, OUT_F], F32, kind="ExternalOutput").ap()

    xt_r = xt_d.rearrange("(k p) b -> p k b", p=P)           # [128, KT, B]
    basist_r = basist_d.rearrange("(k p) h -> p k h", p=P)   # [128, KT, H]
    out_r = out_d.rearrange("(t p) o -> t p o", p=P)         # [BT, 128, O]

    with tile.TileContext(nc) as tc:
        with ExitStack() as ctx:
            const = ctx.enter_context(tc.tile_pool(name="const", bufs=1))
            xpool = ctx.enter_context(tc.tile_pool(name="xp", bufs=3))
            opool = ctx.enter_context(tc.tile_pool(name="op", bufs=3))
            psum1 = ctx.enter_context(tc.tile_pool(name="ps1", bufs=1, space="PSUM"))
            psum2 = ctx.enter_context(tc.tile_pool(name="ps2", bufs=4, space="PSUM"))

            # ---- parameters ----
            basist_sb = const.tile([P, KT, HARM], F32R)
            nc.sync.dma_start(basist_sb[:], basist_r[:])

            phaset_sb = const.tile([P, OUT_F], F32)
            nc.sync.dma_start(phaset_sb[:], phaset_d[:])
            ampt_sb = const.tile([P, OUT_F], F32)
            nc.sync.dma_start(ampt_sb[:], ampt_d[:])

            bias_sb = const.tile([P, 1], F32)
            nc.gpsimd.memset(bias_sb[:], math.pi / 2)
            cost_sb = const.tile([P, OUT_F], F32)
            nc.scalar.activation(
                cost_sb[:], phaset_sb[:], mybir.ActivationFunctionType.Sin,
                bias=bias_sb[:],
            )
            wt_sb = const.tile([P, OUT_F], F32)  # w.T = amp.T * cos(phase.T)
            nc.vector.tensor_mul(wt_sb[:], cost_sb[:], ampt_sb[:])

            # ---- GEMM1: resonanceT[h, b] = sum_k basisT[k,h] * xT[k,b] ----
            ps_res = [
                psum1.tile([P, NCHUNK], F32, tag=f"psr{c}", name=f"psr{c}")
                for c in range(BC)
            ]
            for g in range(NG):
                xg = xpool.tile([P, KG, B], F32R)
                nc.sync.dma_start(xg[:], xt_r[:, g * KG:(g + 1) * KG, :])
                for j in range(KG):
                    k = g * KG + j
                    for c in range(BC):
                        nc.tensor.matmul(
                            ps_res[c][:],
                            lhsT=basist_sb[:, k, :],
                            rhs=xg[:, j, c * NCHUNK:(c + 1) * NCHUNK],
                            start=(k == 0),
                            stop=(k == KT - 1),
                        )

            resont_sb = const.tile([P, B], F32)
            for c in range(BC):
                eng = nc.vector if c % 2 == 0 else nc.scalar
                if eng is nc.vector:
                    eng.tensor_copy(resont_sb[:, c * NCHUNK:(c + 1) * NCHUNK], ps_res[c][:])
                else:
                    eng.copy(resont_sb[:, c * NCHUNK:(c + 1) * NCHUNK], ps_res[c][:])

            # ---- GEMM2: out[b, o] = sum_h resonanceT[h, b] * wT[h, o] ----
            for bt in range(BT):
                og = opool.tile([P, OUT_F], F32)
                for oc in range(OC):
                    ps = psum2.tile([P, NCHUNK], F32)
                    nc.tensor.matmul(
                        ps[:],
                        lhsT=resont_sb[:, bt * P:(bt + 1) * P],
                        rhs=wt_sb[:, oc * NCHUNK:(oc + 1) * NCHUNK],
                        start=True,
                        stop=True,
                    )
                    if oc % 2 == 0:
                        nc.vector.tensor_copy(og[:, oc * NCHUNK:(oc + 1) * NCHUNK], ps[:])
                    else:
                        nc.scalar.copy(og[:, oc * NCHUNK:(oc + 1) * NCHUNK], ps[:])
                nc.scalar.dma_start(out_r[bt], og[:])

    nc.compile()
    return nc


_NC = None


def _get_nc():
    global _NC
    if _NC is None:
        _NC = _build()
    return _NC


def _prep_in_maps(x, basis, phase, amp):
    x = np.ascontiguousarray(x, dtype=np.float32)
    xt = np.ascontiguousarray(x.T)                          # [IN_F, B_FULL]
    basist = np.ascontiguousarray(basis.T, dtype=np.float32)   # [IN_F, H]
    phaset = np.ascontiguousarray(phase.T, dtype=np.float32)   # [H, OUT_F]
    ampt = np.ascontiguousarray(amp.T, dtype=np.float32)       # [H, OUT_F]
    in_maps = []
    for c in range(N_CORES):
        in_maps.append({
            "xt": np.ascontiguousarray(xt[:, c * B:(c + 1) * B]),
            "basist": basist,
            "phaset": phaset,
            "ampt": ampt,
        })
    return in_maps


def _run(inputs, **spmd_kwargs):
    nc = _get_nc()
    in_maps = _prep_in_maps(
        inputs["x"], inputs["basis"], inputs["phase"], inputs["amp"]
    )
    res = run_bass_kernel_spmd(nc, in_maps, list(range(N_CORES)), **spmd_kwargs)
    out = np.concatenate([res.results[c]["out"] for c in range(N_CORES)], axis=0)
    return out, res


def kernel(**inputs) -> np.ndarray:
    out, _ = _run(inputs)
    return out
